# revision 22
# baseline (speedup 1.0000x reference)
"""GCN 3-layer kernel for Trainium2, 8-core SPMD — v2.1.

Math (per layer, PyG GCN with self-loops), normalization fully factorized
into phase-A epilogues so aggregation masks are exact 0/1:
    t_l   = dinv^k * (h @ W_l)          k=1 first layer, 2 after (folds the
                                        deferred dst-side dinv of the
                                        previous aggregation)
    agg_l[f,d] = sum_{e: dst=d} t_l[src_e] + t_l[d]     (self-loop = extra
                                        identity matmul per window)
    out   = relu(dinv_d * agg_3 + b3)   (dst scale only on the last layer)

Distribution: dst-sharded, 6272 nodes/core.  The exchange is chunked into
4 window-aligned AllGathers (13/12/12/12 windows); SWDGE queue q gathers
messages whose src lies in chunk q, so each queue's Q7 core-pair starts as
soon as its chunk's collective lands.  Phase A of layer l+1 and its
collectives are emitted inside layer l's window loop, hiding the exchange
under the gather phase.  Masks are 0/1, built on DVE with batched
single-port tensor_tensor is_equal (never locks the GpSimd SBUF port).
Aggregation matmuls produce agg^T = [feat, dst] directly (lhsT=messages,
rhs=mask): no transposes anywhere; agg^T slices are the next layer's lhsT.

Per-(window,queue) slot counts are padded to the max over cores so the
instruction stream is core-independent; idx/mask tables are per-core data.
"""

import numpy as np

N_NODES = 50000
N_CORES = 8
PER_CORE = 6272            # 49 * 128
N_PAD = PER_CORE * N_CORES # 50176
N_WIN = PER_CORE // 128    # 49
N_Q = 4                    # SWDGE queues == src window-chunks
CH0 = [0, 13, 25, 37, 49]  # chunk window boundaries (13/12/12/12)
QPC = [128 * (CH0[j + 1] - CH0[j]) for j in range(N_Q)]  # rows/core/chunk
QROWS = [N_CORES * r for r in QPC]                       # table rows/chunk
F = 128                    # feature width (layer3 padded 64->128)
F_OUT = 64
CALL_BLOCKS = 16           # 128-slot blocks per dma_gather call
GBUFS = 4                  # m-tile buffers per queue (rotation depth)
MASK_K = 8                 # masks per batched DVE is_equal
LOOKAHEAD = 8              # windows of gather coverage emitted ahead

F16 = np.float16


def _wrap_idx16(idx: np.ndarray) -> np.ndarray:
    """Wrap a flat int16 index stream into the [128, n/16] layout dma_gather
    expects (element i at [i%16, i//16], replicated across the 8 groups of
    16 partitions so any queue's Q7 pair finds them)."""
    n = len(idx)
    assert n % 128 == 0
    cols = n // 16
    out = np.empty((128, cols), np.int16)
    w = idx.reshape(cols, 16).T  # [16, cols]
    for g in range(8):
        out[g * 16:(g + 1) * 16, :] = w
    return out


def _chunk_of_win(w):
    return (w >= CH0[1]).astype(np.int64) + (w >= CH0[2]) + (w >= CH0[3])


# Late-chunk collectives land later: give their queues less gather work.
CHUNK_LOAD = np.array([1.055, 1.035, 1.0, 0.91])


def _balance(src, dst):
    """Permute nodes -> slots: chunks balanced by outdegree (weighted so
    late-starting queues get less), cores balanced by in-degree 4-vectors,
    windows balanced per (core, chunk) by in-degree-per-src-chunk so the
    max-over-cores SPMD padding nearly vanishes.  Returns slot_of[node]."""
    outdeg = np.bincount(src, minlength=N_PAD)
    # phase 0a: nodes -> chunks (capacity 8*QPC[j]) by outdegree, greedy
    order = np.argsort(-outdeg, kind="stable")
    cap = np.array([8 * QPC[j] for j in range(N_Q)])
    tgt = CHUNK_LOAD * outdeg.sum() / CHUNK_LOAD.sum()
    csum = np.zeros(N_Q)
    cnt = np.zeros(N_Q, np.int64)
    chunk_of = np.empty(N_PAD, np.int64)
    for n in order:
        ratio = np.where(cnt < cap, csum / tgt, np.inf)
        j = int(np.argmin(ratio))
        chunk_of[n] = j
        csum[j] += outdeg[n]
        cnt[j] += 1
    # in-degree by src chunk (fixed from here on)
    qdeg = np.zeros((N_PAD, N_Q), np.int64)
    np.add.at(qdeg, (dst, chunk_of[src]), 1)
    # phase 0b: within chunk, nodes -> cores (cap QPC[j]) balancing 4-vectors
    # phase 1: within (core, chunk), nodes -> windows (cap 128) same way
    slot_of = np.empty(N_PAD, np.int64)
    for j in range(N_Q):
        nodes = np.where(chunk_of == j)[0]
        nodes = nodes[np.argsort(-qdeg[nodes].sum(1), kind="stable")]
        S = np.zeros((N_CORES, N_Q))
        ccnt = np.zeros(N_CORES, np.int64)
        core_pick = np.empty(len(nodes), np.int64)
        ctgt = qdeg[nodes].sum(0) / N_CORES + 1e-9
        for i, n in enumerate(nodes):
            score = ((S + qdeg[n]) / ctgt).max(1)
            score[ccnt >= QPC[j]] = np.inf
            c = int(np.argmin(score))
            core_pick[i] = c
            S[c] += qdeg[n]
            ccnt[c] += 1
        nw = CH0[j + 1] - CH0[j]
        for c in range(N_CORES):
            sub = nodes[core_pick == c]
            W = np.zeros((nw, N_Q))
            wcnt = np.zeros(nw, np.int64)
            wtgt = qdeg[sub].sum(0) / nw + 1e-9
            pos = np.empty(len(sub), np.int64)
            for i, n in enumerate(sub):
                score = ((W + qdeg[n]) / wtgt).max(1)
                score[wcnt >= 128] = np.inf
                w = int(np.argmin(score))
                pos[i] = w * 128 + wcnt[w]
                W[w] += qdeg[n]
                wcnt[w] += 1
            slot_of[sub] = c * PER_CORE + (CH0[j] + pos // 128) * 128 + pos % 128
    return slot_of


def _preprocess(edge_index: np.ndarray):
    """Host-side graph prep: degree norm + static SPMD block structure +
    per-core index/mask-value tables."""
    src0 = np.asarray(edge_index[0], np.int64)
    dst0 = np.asarray(edge_index[1], np.int64)
    deg = np.bincount(dst0, minlength=N_NODES).astype(np.float64) + 1.0
    dinv = (1.0 / np.sqrt(deg)).astype(np.float32)

    slot_of = _balance(src0, dst0)
    src = slot_of[src0]
    dst = slot_of[dst0]
    dinv_pad = np.ones(N_PAD, np.float32)
    dinv_pad[slot_of[:N_NODES]] = dinv

    core_of = dst // PER_CORE
    w_of = (dst % PER_CORE) // 128
    d128 = (dst % 128).astype(np.float32)
    # src-side: queue = window-chunk of the src node; idx into chunk table
    src_core = src // PER_CORE
    src_w = (src % PER_CORE) // 128
    src_d = src % 128
    q_of = _chunk_of_win(src_w)
    ch0 = np.array(CH0, np.int64)
    qpc = np.array(QPC, np.int64)
    idx_in_q = src_core * qpc[q_of] + (src_w - ch0[q_of]) * 128 + src_d

    # counts per (core, window, queue); pad to max over cores
    counts = np.zeros((N_CORES, N_WIN, N_Q), np.int64)
    np.add.at(counts, (core_of, w_of, q_of), 1)
    T = counts.max(axis=0)  # [N_WIN, N_Q]

    # static per-queue layout: window w occupies slots [woff[w], woff[w]+T[w,q])
    woff = np.zeros((N_WIN + 1, N_Q), np.int64)
    woff[1:] = np.cumsum(T, axis=0)
    slots_q = [int(-(-woff[N_WIN, q] // 128) * 128) for q in range(N_Q)]
    nb_q = [s // 128 for s in slots_q]

    # slot -> window map (static, -1 for tail pad)
    slot_win = []
    for q in range(N_Q):
        sw = np.full(slots_q[q], -1, np.int64)
        for w in range(N_WIN):
            sw[woff[w, q]:woff[w, q] + T[w, q]] = w
        slot_win.append(sw)

    # global window-major mask/matmul plan: PLAN[w] = [(q, block, col), ...]
    plan = [[] for _ in range(N_WIN)]
    mask_specs = []  # (q, block, w) per mask column
    for w in range(N_WIN):
        for q in range(N_Q):
            if T[w, q] == 0:
                continue
            b0 = int(woff[w, q] // 128)
            b1 = int((woff[w, q] + T[w, q] - 1) // 128)
            for b in range(b0, b1 + 1):
                col = len(mask_specs)
                mask_specs.append((q, b, w))
                plan[w].append((q, b, col))
    nm = len(mask_specs)

    # per-core slot data: idx16 (gather index) + dl (dst-local or 999)
    order = np.lexsort((w_of, q_of, core_of))
    key = (core_of * N_Q + q_of) * N_WIN + w_of
    key_s = key[order]
    idx_s, d128_s = idx_in_q[order], d128[order]
    bounds = np.searchsorted(key_s, np.arange(N_CORES * N_Q * N_WIN + 1))

    idx_all = np.zeros((N_CORES, N_Q), object)
    dlq_all = np.zeros((N_CORES, N_Q), object)
    for c in range(N_CORES):
        for q in range(N_Q):
            idxs = np.zeros(slots_q[q], np.int16)
            dls = np.full(slots_q[q], 999.0, np.float32)
            for w in range(N_WIN):
                k = (c * N_Q + q) * N_WIN + w
                sl = slice(bounds[k], bounds[k + 1])
                n = bounds[k + 1] - bounds[k]
                o = woff[w, q]
                idxs[o:o + n] = idx_s[sl].astype(np.int16)
                dls[o:o + n] = d128_s[sl]
            idx_all[c, q] = idxs
            dlq_all[c, q] = dls

    # mask value matrix [128, nm] per core
    dl_mat = np.full((N_CORES, 128, nm), 999.0, np.float32)
    for col, (q, b, w) in enumerate(mask_specs):
        sl = slice(128 * b, 128 * b + 128)
        valid = slot_win[q][sl] == w
        for c in range(N_CORES):
            dl_mat[c, :, col] = np.where(valid, dlq_all[c, q][sl], 999.0)

    static = dict(T=T, woff=woff, slots_q=slots_q, nb_q=nb_q,
                  plan=plan, nm=nm, slot_of=slot_of)
    return dinv_pad, static, idx_all, dl_mat


def _build_and_run(inputs_np, dinv_pad, static, idx_all, dl_mat,
                   trace=False, sim=False):
    import concourse.bacc as bacc
    import concourse.mybir as mybir
    from concourse.tile import TileContext
    from concourse import bass, bass_utils, library_config

    x = np.asarray(inputs_np["x"], np.float32)
    Ws = [np.asarray(inputs_np[k], np.float32) for k in ("W1", "W2", "W3")]
    bs = [np.asarray(inputs_np[k], np.float32) for k in ("b1", "b2", "b3")]
    assert all(np.all(b == 0.0) for b in bs), "nonzero biases not folded yet"
    W3p = np.zeros((F, F), np.float32)
    W3p[:, :F_OUT] = Ws[2]
    Ws[2] = W3p

    slots_q, nb_q, plan, nm = (static["slots_q"], static["nb_q"],
                               static["plan"], static["nm"])
    woff = static["woff"]
    ncalls = [-(-nb // CALL_BLOCKS) for nb in nb_q]

    nc = bacc.Bacc("TRN2", target_bir_lowering=False, debug=False,
                   num_devices=N_CORES, num_swdge_queues=N_Q)
    dt = mybir.dt

    # ---- kernel I/O -----------------------------------------------------
    t_xT = nc.dram_tensor("xT_own", [128, PER_CORE], dt.float32, kind="ExternalInput")
    t_W = [nc.dram_tensor(f"W{i+1}m", [F, F], dt.float32, kind="ExternalInput") for i in range(3)]
    t_dinv1 = nc.dram_tensor("dinv1", [128, N_WIN], dt.float32, kind="ExternalInput")
    t_dinv2 = nc.dram_tensor("dinv2", [128, N_WIN], dt.float32, kind="ExternalInput")
    t_iota = nc.dram_tensor("iota8", [128, MASK_K * 128], dt.float16, kind="ExternalInput")
    t_ident = nc.dram_tensor("ident", [128, 128], dt.float16, kind="ExternalInput")
    t_idx = [nc.dram_tensor(f"idx_q{q}", [128, slots_q[q] // 16], dt.int16,
                            kind="ExternalInput") for q in range(N_Q)]
    t_dl = nc.dram_tensor("dl_all", [128, nm], dt.float16, kind="ExternalInput")
    t_out = nc.dram_tensor("h_out", [PER_CORE, F_OUT], dt.float32, kind="ExternalOutput")

    with TileContext(nc) as tc:
        nc.gpsimd.load_library(library_config.mlp)
        with tc.tile_pool(name="const", bufs=1) as cpool, \
             tc.tile_pool(name="state", bufs=1) as spool, \
             tc.tile_pool(name="gath", bufs=2) as gpool, \
             tc.tile_pool(name="mask", bufs=4) as mpool, \
             tc.tile_pool(name="psA", bufs=2, space="PSUM") as psA, \
             tc.tile_pool(name="psB", bufs=3, space="PSUM") as psB, \
             tc.tile_pool(name="dram", bufs=1, space="DRAM") as dpool:


            # Tiny collective first: the CC bootstrap barrier (which absorbs
            # core-launch skew) runs during the preamble instead of gating
            # the first real exchange.
            boot_in = dpool.tile([128, F], dt.float16, name="boot_in")
            boot_out = dpool.tile([128 * N_CORES, F], dt.float16,
                                  addr_space="Shared", name="boot_out")
            nc.gpsimd.collective_compute(
                "AllGather", mybir.AluOpType.bypass,
                replica_groups=[list(range(N_CORES))],
                ins=[boot_in.opt()], outs=[boot_out.opt()])

            # ---- constants ----
            c_W = [cpool.tile([F, F], dt.float32, tag=f"W{i}", name=f"cW{i}") for i in range(3)]
            c_dinv1 = cpool.tile([128, N_WIN], dt.float32, tag="dinv1", name="dinv1")
            c_dinv2 = cpool.tile([128, N_WIN], dt.float32, tag="dinv2", name="dinv2")
            c_iota = cpool.tile([128, MASK_K, 128], dt.float16, tag="iota", name="iota")
            c_ident = cpool.tile([128, 128], dt.float16, tag="ident", name="ident")
            c_idx = [cpool.tile([128, slots_q[q] // 16], dt.int16, tag=f"idx{q}",
                                name=f"idx{q}") for q in range(N_Q)]
            c_dl = cpool.tile([128, nm], dt.float16, tag="dl", name="dl")
            for i in range(3):
                nc.sync.dma_start(c_W[i][:], t_W[i][:])
            nc.sync.dma_start(c_dinv1[:], t_dinv1[:])
            nc.sync.dma_start(c_dinv2[:], t_dinv2[:])
            nc.sync.dma_start(c_iota[:], t_iota[:].rearrange("p (k f) -> p k f", k=MASK_K))
            nc.sync.dma_start(c_ident[:], t_ident[:])
            for q in range(N_Q):
                nc.sync.dma_start(c_idx[q][:], t_idx[q][:])
            nc.sync.dma_start(c_dl[:], t_dl[:])

            # ---- persistent state ----
            hT = [spool.tile([128, PER_CORE], dt.float32, tag="hT_a", name="hT_a"),
                  spool.tile([128, PER_CORE], dt.float32, tag="hT_b", name="hT_b")]
            nc.sync.dma_start(hT[0][:], t_xT[:])
            y_sb = [spool.tile([128, N_WIN, F], dt.float16, tag="y_a", name="y_a"),
                    spool.tile([128, N_WIN, F], dt.float16, tag="y_b", name="y_b")]
            out_sb = spool.tile([128, N_WIN, F_OUT], dt.float32, tag="out_sb", name="out_sb")

            t_fulls = [[dpool.tile([QROWS[q], F], dt.float16, addr_space="Shared",
                                   name=f"t_full{l}_{q}") for q in range(N_Q)]
                       for l in range(3)]
            ag_ins = [[dpool.tile([QPC[q], F], dt.float16, name=f"ag_in{l}_{q}")
                       for q in range(N_Q)]
                      for l in range(3)]

            def phase_a(l, w):
                """t_l window w: matmul + ACT scale into y_sb[l%2]."""
                h_in = hT[l % 2]
                c_scale = c_dinv1 if l == 0 else c_dinv2
                ps = psA.tile([128, F], dt.float32, tag="psA", space="PSUM")
                nc.tensor.matmul(ps[:], lhsT=h_in[:, w * 128:(w + 1) * 128],
                                 rhs=c_W[l][:], start=True, stop=True)
                nc.scalar.mul(y_sb[l % 2][:, w, :], ps[:], c_scale[:, w:w + 1])

            def exchange(l, j):
                """ag DMA + AllGather for layer l, chunk j (after phase A
                windows [CH0[j], CH0[j+1]) of layer l are emitted)."""
                nc.sync.dma_start(
                    ag_ins[l][j][:].rearrange("(t p) f -> p t f", p=128),
                    y_sb[l % 2][:, CH0[j]:CH0[j + 1], :])
                nc.gpsimd.collective_compute(
                    "AllGather", mybir.AluOpType.bypass,
                    replica_groups=[list(range(N_CORES))],
                    ins=[ag_ins[l][j].opt()], outs=[t_fulls[l][j].opt()])

            # layer 0 phase A + all exchanges up front
            for j in range(N_Q):
                for w in range(CH0[j], CH0[j + 1]):
                    phase_a(0, w)
                exchange(0, j)

            # ---- gather emission machinery (shared across layers so queue
            # q can flow from layer l straight into layer l+1's calls once
            # cc(l+1, q) has been emitted: no idle Q7 at layer boundaries) --
            # last real window covered by each call (for prefetch gating)
            call_wend = []
            for q in range(N_Q):
                ends = []
                for r in range(ncalls[q]):
                    last = min((r + 1) * CALL_BLOCKS * 128,
                               int(woff[N_WIN, q])) - 1
                    ends.append(int(np.searchsorted(
                        woff[:, q], last, side="right")) - 1)
                call_wend.append(ends)

            gstate = [{"m": {}, "next": [0] * N_Q} for _ in range(3)]

            def emit_call(l, q):
                st = gstate[l]
                r = st["next"][q]
                if r >= ncalls[q]:
                    return False
                n_i = min(CALL_BLOCKS * 128, slots_q[q] - CALL_BLOCKS * 128 * r)
                nb = n_i // 128
                mt = gpool.tile([128, CALL_BLOCKS, F], dt.float16,
                                tag=f"m{q}", name=f"m{q}")
                st["m"][(q, r)] = mt
                nc.gpsimd.dma_gather(
                    out_ap=mt[:, :nb, :],
                    in_ap=t_fulls[l][q][:],
                    idxs_ap=c_idx[q][:, r * CALL_BLOCKS * 8:
                                     r * CALL_BLOCKS * 8 + n_i // 16],
                    num_idxs=n_i, num_idxs_reg=n_i, elem_size=F,
                    queue_num=q, single_packet=False)
                st["next"][q] += 1
                return True

            def cover_upto(l, w_need):
                """Current-layer coverage through w_need (+ lookahead);
                queue 3 deferred (its collective lands last)."""
                for q in range(N_Q):
                    w_tgt = min(w_need + LOOKAHEAD - (8 if q == 3 else 0),
                                N_WIN - 1)
                    w_tgt = max(w_tgt, w_need)
                    need_slot = int(woff[w_tgt + 1, q])
                    while (gstate[l]["next"][q] < ncalls[q]
                           and gstate[l]["next"][q] * CALL_BLOCKS * 128 < need_slot):
                        emit_call(l, q)

            def prefetch_next(l, w):
                """Emit next-layer calls once (a) that queue's collective has
                been emitted and (b) the m-tile buffer being reused has had
                its consumers emitted (avoids Pool head-of-line WAR stalls)."""
                if l >= 2:
                    return
                for q in range(N_Q):
                    if w < CH0[q + 1] - 1:
                        continue
                    r = gstate[l + 1]["next"][q]
                    if r >= ncalls[q]:
                        continue
                    prev = ncalls[q] - GBUFS + r
                    if prev >= 0 and w <= call_wend[q][prev]:
                        continue
                    emit_call(l + 1, q)

            for layer in range(3):
                h_nx = hT[(layer + 1) % 2]
                y_cur = y_sb[layer % 2]
                m_tiles = gstate[layer]["m"]

                # ---- masks ----
                mask_tiles = {}

                def ensure_mask(col, mask_tiles=mask_tiles):
                    j = col // MASK_K
                    if j not in mask_tiles:
                        k0 = j * MASK_K
                        kn = min(MASK_K, nm - k0)
                        S = mpool.tile([128, MASK_K, 128], dt.float16, tag="S",
                                       name="S")
                        nc.vector.tensor_tensor(
                            out=S[:, :kn, :], in0=c_iota[:, :kn, :],
                            in1=c_dl[:, k0:k0 + kn, None].broadcast_to([128, kn, 128]),
                            op=mybir.AluOpType.is_equal)
                        mask_tiles[j] = S
                    return mask_tiles[j], col % MASK_K

                # ---- window loop: phase B(layer) [+ phase A(layer+1) + cc] ----
                for w in range(N_WIN):
                    cover_upto(layer, w)
                    entries = plan[w]
                    if layer < 2:
                        ps = psB.tile([128, F], dt.float32, tag="agg", space="PSUM")
                        nc.tensor.matmul(ps[:], lhsT=y_cur[:, w, :], rhs=c_ident[:],
                                         start=True, stop=not entries)
                        for i, (q, b, col) in enumerate(entries):
                            S, k = ensure_mask(col)
                            r = b // CALL_BLOCKS
                            mt = m_tiles[(q, r)]
                            nc.tensor.matmul(
                                ps[:], lhsT=mt[:, b - r * CALL_BLOCKS, :],
                                rhs=S[:, k, :],
                                start=False, stop=i == len(entries) - 1)
                        nc.scalar.copy(h_nx[:, w * 128:(w + 1) * 128], ps[:])
                        phase_a(layer + 1, w)
                        for j in range(N_Q):
                            if w == CH0[j + 1] - 1:
                                exchange(layer + 1, j)
                    else:
                        ps = psB.tile([128, F_OUT], dt.float32, tag="agg3", space="PSUM")
                        nc.tensor.matmul(ps[:], lhsT=c_ident[:],
                                         rhs=y_cur[:, w, :F_OUT],
                                         start=True, stop=not entries)
                        for i, (q, b, col) in enumerate(entries):
                            S, k = ensure_mask(col)
                            r = b // CALL_BLOCKS
                            mt = m_tiles[(q, r)]
                            nc.tensor.matmul(
                                ps[:], lhsT=S[:, k, :],
                                rhs=mt[:, b - r * CALL_BLOCKS, :F_OUT],
                                start=False, stop=i == len(entries) - 1)
                        nc.scalar.activation(
                            out_sb[:, w, :], ps[:],
                            mybir.ActivationFunctionType.Relu,
                            bias=0.0, scale=c_dinv1[:, w:w + 1])
                    prefetch_next(layer, w)
            nc.sync.dma_start(
                t_out[:].rearrange("(t p) f -> p t f", p=128), out_sb[:])

    nc.compile()

    # ---- per-core inputs ----
    slot_of = static["slot_of"]
    xT_all = np.zeros((128, N_PAD), np.float32)
    xT_all[:, slot_of[:N_NODES]] = x.T
    iota8 = np.tile(np.arange(128, dtype=np.float32), MASK_K)
    iota8 = np.broadcast_to(iota8.astype(F16), (128, MASK_K * 128)).copy()
    ident = np.eye(128, dtype=F16)
    in_maps = []
    for c in range(N_CORES):
        rows = slice(c * PER_CORE, (c + 1) * PER_CORE)
        din = dinv_pad[rows].reshape(N_WIN, 128).T.copy()  # [128, N_WIN]
        in_map = {
            "xT_own": np.ascontiguousarray(xT_all[:, rows]),
            "dinv1": din,
            "dinv2": (din * din),
            "iota8": iota8.copy(),
            "ident": ident.copy(),
            "dl_all": dl_mat[c].astype(F16),
        }
        for q in range(N_Q):
            in_map[f"idx_q{q}"] = _wrap_idx16(idx_all[c, q])
        for i in range(3):
            in_map[f"W{i+1}m"] = Ws[i].copy()
        in_maps.append(in_map)

    if sim:
        from concourse.bass_interp import MultiCoreSim
        mcs = MultiCoreSim(nc, num_cores=N_CORES, trace=False,
                           require_finite=False, require_nnan=False)
        for ci, core in enumerate(mcs.cores.values()):
            for k, v in in_maps[ci].items():
                core.tensor(k)[:] = v
        mcs.simulate(check_with_hw=False)
        outs = [np.asarray(core.tensor("h_out"))
                for core in mcs.cores.values()]
        res = None
    else:
        res = bass_utils.run_bass_kernel_spmd(
            nc, in_maps, core_ids=list(range(N_CORES)), trace=trace)
        outs = [r["h_out"] for r in res.results]
    full = np.concatenate(outs, axis=0)[slot_of[:N_NODES]]
    return full, res


def kernel(**inputs) -> np.ndarray:
    edge_index = np.asarray(inputs["edge_index"])
    prep = _preprocess(edge_index)
    out, _ = _build_and_run(inputs, *prep)
    return out


# revision 24
# speedup vs baseline: 1.0064x; 1.0064x over previous
"""GCN 3-layer kernel for Trainium2, 8-core SPMD.

Math (per layer, PyG GCN with self-loops), normalization fully factorized
into phase-A epilogues so aggregation masks are exact 0/1:
    t_l   = dinv^k * (h @ W_l)          k=1 first layer, 2 after (folds the
                                        deferred dst-side dinv of the
                                        previous aggregation)
    agg_l[f,d] = sum_{e: dst=d} t_l[src_e] + t_l[d]     (self-loop = extra
                                        identity matmul per window)
    out   = relu(dinv_d * agg_3 + b3)   (dst scale only on the last layer)

Distribution: dst-sharded, 6272 nodes/core.  The exchange is chunked into
4 window-aligned AllGathers (13/12/12/12 windows); SWDGE queue q gathers
messages whose src lies in chunk q, so each queue's Q7 core-pair starts as
soon as its chunk's collective lands.  Phase A of layer l+1 and its
collectives are emitted inside layer l's window loop, hiding the exchange
under the gather phase; next-layer gather calls are prefetched across the
layer boundary once their collective and m-tile buffer are safe.  Masks
are 0/1, built on DVE with batched single-port tensor_tensor is_equal
(never locks the GpSimd SBUF port the Q7 descriptor generators need).
Aggregation matmuls produce agg^T = [feat, dst] directly (lhsT=messages,
rhs=mask): no transposes anywhere; agg^T slices are the next layer's lhsT.

A host-side balancer permutes nodes so (a) chunks carry outdegree loads
weighted against their collective's landing time, (b) per-(window,queue)
in-degree counts are near-equal across cores, shrinking the max-over-cores
SPMD padding to <1%.  Instruction stream is core-independent; idx/mask
tables are per-core data.  Bottleneck: Q7 SWDGE descriptor generation
(~6ns/row engine time, ~2x gen/drain overlap across queue pairs).
"""

import numpy as np

N_NODES = 50000
N_CORES = 8
PER_CORE = 6272            # 49 * 128
N_PAD = PER_CORE * N_CORES # 50176
N_WIN = PER_CORE // 128    # 49
N_Q = 4                    # SWDGE queues == src window-chunks
CH0 = [0, 13, 25, 37, 49]  # chunk window boundaries (13/12/12/12)
QPC = [128 * (CH0[j + 1] - CH0[j]) for j in range(N_Q)]  # rows/core/chunk
QROWS = [N_CORES * r for r in QPC]                       # table rows/chunk
F = 128                    # feature width (layer3 padded 64->128)
F_OUT = 64
CALL_BLOCKS = 16           # 128-slot blocks per dma_gather call
GBUFS = 4                  # m-tile buffers per queue (rotation depth)
MASK_K = 8                 # masks per batched DVE is_equal
LOOKAHEAD = 8              # windows of gather coverage emitted ahead

F16 = np.float16


def _wrap_idx16(idx: np.ndarray) -> np.ndarray:
    """Wrap a flat int16 index stream into the [128, n/16] layout dma_gather
    expects (element i at [i%16, i//16], replicated across the 8 groups of
    16 partitions so any queue's Q7 pair finds them)."""
    n = len(idx)
    assert n % 128 == 0
    cols = n // 16
    out = np.empty((128, cols), np.int16)
    w = idx.reshape(cols, 16).T  # [16, cols]
    for g in range(8):
        out[g * 16:(g + 1) * 16, :] = w
    return out


def _chunk_of_win(w):
    return (w >= CH0[1]).astype(np.int64) + (w >= CH0[2]) + (w >= CH0[3])


# Late-chunk collectives land later: give their queues less gather work.
CHUNK_LOAD = np.array([1.055, 1.035, 1.0, 0.91])


def _balance(src, dst):
    """Permute nodes -> slots: chunks balanced by outdegree (weighted so
    late-starting queues get less), cores balanced by in-degree 4-vectors,
    windows balanced per (core, chunk) by in-degree-per-src-chunk so the
    max-over-cores SPMD padding nearly vanishes.  Returns slot_of[node]."""
    outdeg = np.bincount(src, minlength=N_PAD)
    # phase 0a: nodes -> chunks (capacity 8*QPC[j]) by outdegree, greedy
    order = np.argsort(-outdeg, kind="stable")
    cap = np.array([8 * QPC[j] for j in range(N_Q)])
    tgt = CHUNK_LOAD * outdeg.sum() / CHUNK_LOAD.sum()
    csum = np.zeros(N_Q)
    cnt = np.zeros(N_Q, np.int64)
    chunk_of = np.empty(N_PAD, np.int64)
    for n in order:
        ratio = np.where(cnt < cap, csum / tgt, np.inf)
        j = int(np.argmin(ratio))
        chunk_of[n] = j
        csum[j] += outdeg[n]
        cnt[j] += 1
    # in-degree by src chunk (fixed from here on)
    qdeg = np.zeros((N_PAD, N_Q), np.int64)
    np.add.at(qdeg, (dst, chunk_of[src]), 1)
    # phase 0b: within chunk, nodes -> cores (cap QPC[j]) balancing 4-vectors
    # phase 1: within (core, chunk), nodes -> windows (cap 128) same way
    slot_of = np.empty(N_PAD, np.int64)
    for j in range(N_Q):
        nodes = np.where(chunk_of == j)[0]
        nodes = nodes[np.argsort(-qdeg[nodes].sum(1), kind="stable")]
        S = np.zeros((N_CORES, N_Q))
        ccnt = np.zeros(N_CORES, np.int64)
        core_pick = np.empty(len(nodes), np.int64)
        ctgt = qdeg[nodes].sum(0) / N_CORES + 1e-9
        for i, n in enumerate(nodes):
            score = ((S + qdeg[n]) / ctgt).max(1)
            score[ccnt >= QPC[j]] = np.inf
            c = int(np.argmin(score))
            core_pick[i] = c
            S[c] += qdeg[n]
            ccnt[c] += 1
        nw = CH0[j + 1] - CH0[j]
        for c in range(N_CORES):
            sub = nodes[core_pick == c]
            W = np.zeros((nw, N_Q))
            wcnt = np.zeros(nw, np.int64)
            wtgt = qdeg[sub].sum(0) / nw + 1e-9
            pos = np.empty(len(sub), np.int64)
            for i, n in enumerate(sub):
                score = ((W + qdeg[n]) / wtgt).max(1)
                score[wcnt >= 128] = np.inf
                w = int(np.argmin(score))
                pos[i] = w * 128 + wcnt[w]
                W[w] += qdeg[n]
                wcnt[w] += 1
            slot_of[sub] = c * PER_CORE + (CH0[j] + pos // 128) * 128 + pos % 128
    return slot_of


def _preprocess(edge_index: np.ndarray):
    """Host-side graph prep: degree norm + static SPMD block structure +
    per-core index/mask-value tables."""
    src0 = np.asarray(edge_index[0], np.int64)
    dst0 = np.asarray(edge_index[1], np.int64)
    deg = np.bincount(dst0, minlength=N_NODES).astype(np.float64) + 1.0
    dinv = (1.0 / np.sqrt(deg)).astype(np.float32)

    slot_of = _balance(src0, dst0)
    src = slot_of[src0]
    dst = slot_of[dst0]
    dinv_pad = np.ones(N_PAD, np.float32)
    dinv_pad[slot_of[:N_NODES]] = dinv

    core_of = dst // PER_CORE
    w_of = (dst % PER_CORE) // 128
    d128 = (dst % 128).astype(np.float32)
    # src-side: queue = window-chunk of the src node; idx into chunk table
    src_core = src // PER_CORE
    src_w = (src % PER_CORE) // 128
    src_d = src % 128
    q_of = _chunk_of_win(src_w)
    ch0 = np.array(CH0, np.int64)
    qpc = np.array(QPC, np.int64)
    idx_in_q = src_core * qpc[q_of] + (src_w - ch0[q_of]) * 128 + src_d

    # counts per (core, window, queue); pad to max over cores
    counts = np.zeros((N_CORES, N_WIN, N_Q), np.int64)
    np.add.at(counts, (core_of, w_of, q_of), 1)
    T = counts.max(axis=0)  # [N_WIN, N_Q]

    # static per-queue layout: window w occupies slots [woff[w], woff[w]+T[w,q])
    woff = np.zeros((N_WIN + 1, N_Q), np.int64)
    woff[1:] = np.cumsum(T, axis=0)
    slots_q = [int(-(-woff[N_WIN, q] // 128) * 128) for q in range(N_Q)]
    nb_q = [s // 128 for s in slots_q]

    # slot -> window map (static, -1 for tail pad)
    slot_win = []
    for q in range(N_Q):
        sw = np.full(slots_q[q], -1, np.int64)
        for w in range(N_WIN):
            sw[woff[w, q]:woff[w, q] + T[w, q]] = w
        slot_win.append(sw)

    # global window-major mask/matmul plan: PLAN[w] = [(q, block, col), ...]
    plan = [[] for _ in range(N_WIN)]
    mask_specs = []  # (q, block, w) per mask column
    for w in range(N_WIN):
        for q in range(N_Q):
            if T[w, q] == 0:
                continue
            b0 = int(woff[w, q] // 128)
            b1 = int((woff[w, q] + T[w, q] - 1) // 128)
            for b in range(b0, b1 + 1):
                col = len(mask_specs)
                mask_specs.append((q, b, w))
                plan[w].append((q, b, col))
    nm = len(mask_specs)

    # per-core slot data: idx16 (gather index) + dl (dst-local or 999)
    order = np.lexsort((w_of, q_of, core_of))
    key = (core_of * N_Q + q_of) * N_WIN + w_of
    key_s = key[order]
    idx_s, d128_s = idx_in_q[order], d128[order]
    bounds = np.searchsorted(key_s, np.arange(N_CORES * N_Q * N_WIN + 1))

    idx_all = np.zeros((N_CORES, N_Q), object)
    dlq_all = np.zeros((N_CORES, N_Q), object)
    for c in range(N_CORES):
        for q in range(N_Q):
            idxs = np.zeros(slots_q[q], np.int16)
            dls = np.full(slots_q[q], 999.0, np.float32)
            for w in range(N_WIN):
                k = (c * N_Q + q) * N_WIN + w
                sl = slice(bounds[k], bounds[k + 1])
                n = bounds[k + 1] - bounds[k]
                o = woff[w, q]
                idxs[o:o + n] = idx_s[sl].astype(np.int16)
                dls[o:o + n] = d128_s[sl]
            idx_all[c, q] = idxs
            dlq_all[c, q] = dls

    # mask value matrix [128, nm] per core
    dl_mat = np.full((N_CORES, 128, nm), 999.0, np.float32)
    for col, (q, b, w) in enumerate(mask_specs):
        sl = slice(128 * b, 128 * b + 128)
        valid = slot_win[q][sl] == w
        for c in range(N_CORES):
            dl_mat[c, :, col] = np.where(valid, dlq_all[c, q][sl], 999.0)

    static = dict(T=T, woff=woff, slots_q=slots_q, nb_q=nb_q,
                  plan=plan, nm=nm, slot_of=slot_of)
    return dinv_pad, static, idx_all, dl_mat


def _build_and_run(inputs_np, dinv_pad, static, idx_all, dl_mat,
                   trace=False, sim=False):
    import concourse.bacc as bacc
    import concourse.mybir as mybir
    from concourse.tile import TileContext
    from concourse import bass, bass_utils, library_config

    x = np.asarray(inputs_np["x"], np.float32)
    Ws = [np.asarray(inputs_np[k], np.float32) for k in ("W1", "W2", "W3")]
    bs = [np.asarray(inputs_np[k], np.float32) for k in ("b1", "b2", "b3")]
    assert all(np.all(b == 0.0) for b in bs), "nonzero biases not folded yet"
    W3p = np.zeros((F, F), np.float32)
    W3p[:, :F_OUT] = Ws[2]
    Ws[2] = W3p

    slots_q, nb_q, plan, nm = (static["slots_q"], static["nb_q"],
                               static["plan"], static["nm"])
    woff = static["woff"]
    ncalls = [-(-nb // CALL_BLOCKS) for nb in nb_q]

    nc = bacc.Bacc("TRN2", target_bir_lowering=False, debug=False,
                   num_devices=N_CORES, num_swdge_queues=N_Q)
    dt = mybir.dt

    # ---- kernel I/O -----------------------------------------------------
    t_xT = nc.dram_tensor("xT_own", [128, PER_CORE], dt.float32, kind="ExternalInput")
    t_W = [nc.dram_tensor(f"W{i+1}m", [F, F], dt.float32, kind="ExternalInput") for i in range(3)]
    t_dinv1 = nc.dram_tensor("dinv1", [128, N_WIN], dt.float32, kind="ExternalInput")
    t_dinv2 = nc.dram_tensor("dinv2", [128, N_WIN], dt.float32, kind="ExternalInput")
    t_iota = nc.dram_tensor("iota8", [128, MASK_K * 128], dt.float16, kind="ExternalInput")
    t_ident = nc.dram_tensor("ident", [128, 128], dt.float16, kind="ExternalInput")
    t_idx = [nc.dram_tensor(f"idx_q{q}", [128, slots_q[q] // 16], dt.int16,
                            kind="ExternalInput") for q in range(N_Q)]
    t_dl = nc.dram_tensor("dl_all", [128, nm], dt.float16, kind="ExternalInput")
    t_out = nc.dram_tensor("h_out", [PER_CORE, F_OUT], dt.float32, kind="ExternalOutput")

    with TileContext(nc) as tc:
        nc.gpsimd.load_library(library_config.mlp)
        with tc.tile_pool(name="const", bufs=1) as cpool, \
             tc.tile_pool(name="state", bufs=1) as spool, \
             tc.tile_pool(name="gath", bufs=2) as gpool, \
             tc.tile_pool(name="mask", bufs=4) as mpool, \
             tc.tile_pool(name="psA", bufs=2, space="PSUM") as psA, \
             tc.tile_pool(name="psB", bufs=3, space="PSUM") as psB, \
             tc.tile_pool(name="dram", bufs=1, space="DRAM") as dpool:


            # Tiny collective first: the CC bootstrap barrier (which absorbs
            # core-launch skew) runs during the preamble instead of gating
            # the first real exchange.
            boot_in = dpool.tile([128, F], dt.float16, name="boot_in")
            boot_out = dpool.tile([128 * N_CORES, F], dt.float16,
                                  addr_space="Shared", name="boot_out")
            nc.gpsimd.collective_compute(
                "AllGather", mybir.AluOpType.bypass,
                replica_groups=[list(range(N_CORES))],
                ins=[boot_in.opt()], outs=[boot_out.opt()])

            # ---- constants ----
            c_W = [cpool.tile([F, F], dt.float32, tag=f"W{i}", name=f"cW{i}") for i in range(3)]
            c_dinv1 = cpool.tile([128, N_WIN], dt.float32, tag="dinv1", name="dinv1")
            c_dinv2 = cpool.tile([128, N_WIN], dt.float32, tag="dinv2", name="dinv2")
            c_iota = cpool.tile([128, MASK_K, 128], dt.float16, tag="iota", name="iota")
            c_ident = cpool.tile([128, 128], dt.float16, tag="ident", name="ident")
            c_idx = [cpool.tile([128, slots_q[q] // 16], dt.int16, tag=f"idx{q}",
                                name=f"idx{q}") for q in range(N_Q)]
            c_dl = cpool.tile([128, nm], dt.float16, tag="dl", name="dl")
            for i in range(3):
                nc.sync.dma_start(c_W[i][:], t_W[i][:])
            nc.sync.dma_start(c_dinv1[:], t_dinv1[:])
            nc.sync.dma_start(c_dinv2[:], t_dinv2[:])
            nc.sync.dma_start(c_iota[:], t_iota[:].rearrange("p (k f) -> p k f", k=MASK_K))
            nc.sync.dma_start(c_ident[:], t_ident[:])
            for q in range(N_Q):
                nc.sync.dma_start(c_idx[q][:], t_idx[q][:])
            nc.sync.dma_start(c_dl[:], t_dl[:])

            # ---- persistent state ----
            hT = [spool.tile([128, PER_CORE], dt.float32, tag="hT_a", name="hT_a"),
                  spool.tile([128, PER_CORE], dt.float32, tag="hT_b", name="hT_b")]
            nc.sync.dma_start(hT[0][:], t_xT[:])
            y_sb = [spool.tile([128, N_WIN, F], dt.float16, tag="y_a", name="y_a"),
                    spool.tile([128, N_WIN, F], dt.float16, tag="y_b", name="y_b")]
            out_sb = spool.tile([128, N_WIN, F_OUT], dt.float32, tag="out_sb", name="out_sb")

            t_fulls = [[dpool.tile([QROWS[q], F], dt.float16, addr_space="Shared",
                                   name=f"t_full{l}_{q}") for q in range(N_Q)]
                       for l in range(3)]
            ag_ins = [[dpool.tile([QPC[q], F], dt.float16, name=f"ag_in{l}_{q}")
                       for q in range(N_Q)]
                      for l in range(3)]

            def phase_a(l, w):
                """t_l window w: matmul + ACT scale into y_sb[l%2]."""
                h_in = hT[l % 2]
                c_scale = c_dinv1 if l == 0 else c_dinv2
                ps = psA.tile([128, F], dt.float32, tag="psA", space="PSUM")
                nc.tensor.matmul(ps[:], lhsT=h_in[:, w * 128:(w + 1) * 128],
                                 rhs=c_W[l][:], start=True, stop=True)
                nc.scalar.mul(y_sb[l % 2][:, w, :], ps[:], c_scale[:, w:w + 1])

            def exchange(l, j):
                """ag DMA + AllGather for layer l, chunk j (after phase A
                windows [CH0[j], CH0[j+1]) of layer l are emitted)."""
                nc.sync.dma_start(
                    ag_ins[l][j][:].rearrange("(t p) f -> p t f", p=128),
                    y_sb[l % 2][:, CH0[j]:CH0[j + 1], :])
                nc.gpsimd.collective_compute(
                    "AllGather", mybir.AluOpType.bypass,
                    replica_groups=[list(range(N_CORES))],
                    ins=[ag_ins[l][j].opt()], outs=[t_fulls[l][j].opt()])

            # layer 0 phase A + all exchanges up front
            for j in range(N_Q):
                for w in range(CH0[j], CH0[j + 1]):
                    phase_a(0, w)
                exchange(0, j)

            # ---- gather emission machinery (shared across layers so queue
            # q can flow from layer l straight into layer l+1's calls once
            # cc(l+1, q) has been emitted: no idle Q7 at layer boundaries) --
            # last real window covered by each call (for prefetch gating)
            call_wend = []
            for q in range(N_Q):
                ends = []
                for r in range(ncalls[q]):
                    last = min((r + 1) * CALL_BLOCKS * 128,
                               int(woff[N_WIN, q])) - 1
                    ends.append(int(np.searchsorted(
                        woff[:, q], last, side="right")) - 1)
                call_wend.append(ends)

            gstate = [{"m": {}, "next": [0] * N_Q} for _ in range(3)]

            def emit_call(l, q):
                st = gstate[l]
                r = st["next"][q]
                if r >= ncalls[q]:
                    return False
                n_i = min(CALL_BLOCKS * 128, slots_q[q] - CALL_BLOCKS * 128 * r)
                nb = n_i // 128
                mt = gpool.tile([128, CALL_BLOCKS, F], dt.float16,
                                tag=f"m{q}", name=f"m{q}")
                st["m"][(q, r)] = mt
                nc.gpsimd.dma_gather(
                    out_ap=mt[:, :nb, :],
                    in_ap=t_fulls[l][q][:],
                    idxs_ap=c_idx[q][:, r * CALL_BLOCKS * 8:
                                     r * CALL_BLOCKS * 8 + n_i // 16],
                    num_idxs=n_i, num_idxs_reg=n_i, elem_size=F,
                    queue_num=q, single_packet=False)
                st["next"][q] += 1
                return True

            def cover_upto(l, w_need):
                """Current-layer coverage through w_need (+ lookahead);
                queue 3 deferred (its collective lands last)."""
                for q in range(N_Q):
                    w_tgt = min(w_need + LOOKAHEAD - (8 if q == 3 else 0),
                                N_WIN - 1)
                    w_tgt = max(w_tgt, w_need)
                    need_slot = int(woff[w_tgt + 1, q])
                    while (gstate[l]["next"][q] < ncalls[q]
                           and gstate[l]["next"][q] * CALL_BLOCKS * 128 < need_slot):
                        emit_call(l, q)

            def prefetch_next(l, w):
                """Emit next-layer calls once (a) that queue's collective has
                been emitted and (b) the m-tile buffer being reused has had
                its consumers emitted (avoids Pool head-of-line WAR stalls)."""
                if l >= 2:
                    return
                for q in range(N_Q):
                    if w < CH0[q + 1] - 1:
                        continue
                    r = gstate[l + 1]["next"][q]
                    if r >= ncalls[q]:
                        continue
                    prev = ncalls[q] - GBUFS + r
                    if prev >= 0 and w <= call_wend[q][prev]:
                        continue
                    emit_call(l + 1, q)

            for layer in range(3):
                h_nx = hT[(layer + 1) % 2]
                y_cur = y_sb[layer % 2]
                m_tiles = gstate[layer]["m"]

                # ---- masks ----
                mask_tiles = {}

                def ensure_mask(col, mask_tiles=mask_tiles):
                    j = col // MASK_K
                    if j not in mask_tiles:
                        k0 = j * MASK_K
                        kn = min(MASK_K, nm - k0)
                        S = mpool.tile([128, MASK_K, 128], dt.float16, tag="S",
                                       name="S")
                        nc.vector.tensor_tensor(
                            out=S[:, :kn, :], in0=c_iota[:, :kn, :],
                            in1=c_dl[:, k0:k0 + kn, None].broadcast_to([128, kn, 128]),
                            op=mybir.AluOpType.is_equal)
                        mask_tiles[j] = S
                    return mask_tiles[j], col % MASK_K

                # ---- window loop: phase B(layer) [+ phase A(layer+1) + cc] ----
                for w in range(N_WIN):
                    cover_upto(layer, w)
                    entries = plan[w]
                    if layer < 2:
                        ps = psB.tile([128, F], dt.float32, tag="agg", space="PSUM")
                        nc.tensor.matmul(ps[:], lhsT=y_cur[:, w, :], rhs=c_ident[:],
                                         start=True, stop=not entries)
                        for i, (q, b, col) in enumerate(entries):
                            S, k = ensure_mask(col)
                            r = b // CALL_BLOCKS
                            mt = m_tiles[(q, r)]
                            nc.tensor.matmul(
                                ps[:], lhsT=mt[:, b - r * CALL_BLOCKS, :],
                                rhs=S[:, k, :],
                                start=False, stop=i == len(entries) - 1)
                        nc.scalar.copy(h_nx[:, w * 128:(w + 1) * 128], ps[:])
                        phase_a(layer + 1, w)
                        for j in range(N_Q):
                            if w == CH0[j + 1] - 1:
                                exchange(layer + 1, j)
                    else:
                        ps = psB.tile([128, F_OUT], dt.float32, tag="agg3", space="PSUM")
                        nc.tensor.matmul(ps[:], lhsT=c_ident[:],
                                         rhs=y_cur[:, w, :F_OUT],
                                         start=True, stop=not entries)
                        for i, (q, b, col) in enumerate(entries):
                            S, k = ensure_mask(col)
                            r = b // CALL_BLOCKS
                            mt = m_tiles[(q, r)]
                            nc.tensor.matmul(
                                ps[:], lhsT=S[:, k, :],
                                rhs=mt[:, b - r * CALL_BLOCKS, :F_OUT],
                                start=False, stop=i == len(entries) - 1)
                        nc.scalar.activation(
                            out_sb[:, w, :], ps[:],
                            mybir.ActivationFunctionType.Relu,
                            bias=0.0, scale=c_dinv1[:, w:w + 1])
                    prefetch_next(layer, w)
            nc.sync.dma_start(
                t_out[:].rearrange("(t p) f -> p t f", p=128), out_sb[:])

    nc.compile()

    # ---- per-core inputs ----
    slot_of = static["slot_of"]
    xT_all = np.zeros((128, N_PAD), np.float32)
    xT_all[:, slot_of[:N_NODES]] = x.T
    iota8 = np.tile(np.arange(128, dtype=np.float32), MASK_K)
    iota8 = np.broadcast_to(iota8.astype(F16), (128, MASK_K * 128)).copy()
    ident = np.eye(128, dtype=F16)
    in_maps = []
    for c in range(N_CORES):
        rows = slice(c * PER_CORE, (c + 1) * PER_CORE)
        din = dinv_pad[rows].reshape(N_WIN, 128).T.copy()  # [128, N_WIN]
        in_map = {
            "xT_own": np.ascontiguousarray(xT_all[:, rows]),
            "dinv1": din,
            "dinv2": (din * din),
            "iota8": iota8.copy(),
            "ident": ident.copy(),
            "dl_all": dl_mat[c].astype(F16),
        }
        for q in range(N_Q):
            in_map[f"idx_q{q}"] = _wrap_idx16(idx_all[c, q])
        for i in range(3):
            in_map[f"W{i+1}m"] = Ws[i].copy()
        in_maps.append(in_map)

    if sim:
        from concourse.bass_interp import MultiCoreSim
        mcs = MultiCoreSim(nc, num_cores=N_CORES, trace=False,
                           require_finite=False, require_nnan=False)
        for ci, core in enumerate(mcs.cores.values()):
            for k, v in in_maps[ci].items():
                core.tensor(k)[:] = v
        mcs.simulate(check_with_hw=False)
        outs = [np.asarray(core.tensor("h_out"))
                for core in mcs.cores.values()]
        res = None
    else:
        res = bass_utils.run_bass_kernel_spmd(
            nc, in_maps, core_ids=list(range(N_CORES)), trace=trace)
        outs = [r["h_out"] for r in res.results]
    full = np.concatenate(outs, axis=0)[slot_of[:N_NODES]]
    return full, res


def kernel(**inputs) -> np.ndarray:
    edge_index = np.asarray(inputs["edge_index"])
    prep = _preprocess(edge_index)
    out, _ = _build_and_run(inputs, *prep)
    return out
